# revision 1
# baseline (speedup 1.0000x reference)
import sys, os
sys.path.insert(0, "/opt/trn_rl_repo")
from contextlib import ExitStack

import numpy as np
import ml_dtypes

import concourse.bass as bass
import concourse.tile as tile
import concourse.masks as masks
from concourse import bacc, mybir
from concourse.bass_utils import run_bass_kernel_spmd

F32 = mybir.dt.float32
BF16 = mybir.dt.bfloat16
I16 = mybir.dt.int16
OP = mybir.AluOpType
ACTF = mybir.ActivationFunctionType

T_FULL, N, E = 8, 20000, 640000
DIN, H, KH = 2, 64, 3
CORES = 8
NLOC = N // CORES            # 2500
NBLK = (NLOC + 127) // 128   # 20
NPAD = NBLK * 128            # 2560
FW = DIN + H                 # 66
ROWE = 128                   # padded table row (elements)

VDT = BF16                   # value dtype for tables / one-hot / scatter matmul


def _npdt(vdt):
    return np.float32 if vdt == F32 else ml_dtypes.bfloat16


def preprocess(x, edge_idx, edge_attr, n_steps, vdt):
    x = np.asarray(x, np.float32)
    ei = np.asarray(edge_idx)
    ea = np.asarray(edge_attr, np.float32)
    npdt = _npdt(vdt)

    src_all, dst_all = ei[:, 0, :], ei[:, 1, :]

    # pass 1: global CBLK (chunks per dst block) and LELL (max out-degree)
    cmax, lmax = 0, 0
    for t in range(n_steps):
        s, d, = src_all[t], dst_all[t]
        for c in range(CORES):
            m = (d // NLOC) == c
            dl = d[m] - c * NLOC
            cnt = np.bincount(dl // 128, minlength=NBLK)
            cmax = max(cmax, int(cnt.max()))
            m2 = (s // NLOC) == c
            ls = s[m2] - c * NLOC
            oc = np.bincount(ls, minlength=NLOC)
            lmax = max(lmax, int(oc.max()))
    cblk = (cmax + 127) // 128
    lell = max(4, lmax)
    nch = NBLK * cblk

    maps = []
    for c in range(CORES):
        idxw = np.zeros((n_steps, 128, nch * 8), np.int16)
        dlq = np.zeros((n_steps, 128, nch), np.float32)
        wq = np.zeros((n_steps, 128, nch), np.float32)
        wel = np.zeros((n_steps, 128, NBLK, lell), np.float32)
        xar = np.zeros((n_steps, 128, NBLK, DIN), np.float32)
        for t in range(n_steps):
            s, d, w = src_all[t], dst_all[t], ea[t]
            m = (d // NLOC) == c
            ss, dd, ww = s[m], d[m] - c * NLOC, w[m]
            b = dd // 128
            loc = (dd % 128).astype(np.float32)
            gsrc = ((ss // NLOC) * NPAD + ss % NLOC).astype(np.int16)
            order = np.argsort(b, kind="stable")
            bs = b[order]
            start = np.searchsorted(bs, np.arange(NBLK))
            pos = np.arange(len(bs)) - start[bs]
            gs = np.zeros((NBLK, cblk * 128), np.int16)
            gl = np.zeros((NBLK, cblk * 128), np.float32)
            gw = np.zeros((NBLK, cblk * 128), np.float32)
            gs[bs, pos] = gsrc[order]
            gl[bs, pos] = loc[order]
            gw[bs, pos] = ww[order]
            # idx wrap: per block, j -> (col=j//16, row=j%16), replicated x8
            iw = gs.reshape(NBLK, cblk * 8, 16).transpose(0, 2, 1)  # [NBLK,16,cblk*8]
            iw = np.tile(iw, (1, 8, 1))                             # [NBLK,128,cblk*8]
            idxw[t] = iw.transpose(1, 0, 2).reshape(128, nch * 8)
            # chunk-column layout: [128, NBLK*cblk], elem (p, b*cblk+cx) = edge cx*128+p
            dlq[t] = gl.reshape(NBLK, cblk, 128).transpose(2, 0, 1).reshape(128, nch)
            wq[t] = gw.reshape(NBLK, cblk, 128).transpose(2, 0, 1).reshape(128, nch)
            # src ELL for degree
            m2 = (s // NLOC) == c
            ls, w2 = s[m2] - c * NLOC, w[m2]
            o2 = np.argsort(ls, kind="stable")
            lss = ls[o2]
            st2 = np.searchsorted(lss, np.arange(NLOC))
            pos2 = np.arange(len(lss)) - st2[lss]
            wel[t, lss % 128, lss // 128, pos2] = w2[o2]
            # x, node-major blocked
            xl = np.zeros((NPAD, DIN), np.float32)
            xl[:NLOC] = x[t, c * NLOC:(c + 1) * NLOC]
            xar[t] = xl.reshape(NBLK, 128, DIN).transpose(1, 0, 2)
        maps.append(dict(idxw=idxw, dl=dlq, w=wq, well=wel, xarr=xar))
    return maps, cblk, lell


def build(n_steps, cblk, lell, vdt):
    nc = bacc.Bacc("TRN2", target_bir_lowering=False, debug=False)
    nch = NBLK * cblk
    nix = cblk * 128

    d_idx = nc.dram_tensor("idxw", [n_steps, 128, nch * 8], I16, kind="ExternalInput")
    d_dl = nc.dram_tensor("dl", [n_steps, 128, nch], F32, kind="ExternalInput")
    d_w = nc.dram_tensor("w", [n_steps, 128, nch], F32, kind="ExternalInput")
    d_wel = nc.dram_tensor("well", [n_steps, 128, NBLK, lell], F32, kind="ExternalInput")
    d_x = nc.dram_tensor("xarr", [n_steps, 128, NBLK, DIN], F32, kind="ExternalInput")
    d_W = {g: nc.dram_tensor(f"W{g}", [KH, FW, H], F32, kind="ExternalInput") for g in "ruc"}
    d_b = {g: nc.dram_tensor(f"b{g}", [1, H], F32, kind="ExternalInput") for g in "ruc"}
    d_iota = nc.dram_tensor("iota", [128, 128], vdt, kind="ExternalInput")
    d_out = nc.dram_tensor("h_out", [128, NBLK, H], F32, kind="ExternalOutput")

    with tile.TileContext(nc) as tc, ExitStack() as ctx:
        const = ctx.enter_context(tc.tile_pool(name="const", bufs=1))
        sb = ctx.enter_context(tc.tile_pool(name="sb", bufs=2))
        gpool = ctx.enter_context(tc.tile_pool(name="gath", bufs=4))
        dpool = ctx.enter_context(tc.tile_pool(name="oneh", bufs=8))
        spool = ctx.enter_context(tc.tile_pool(name="small", bufs=4))
        ppool = ctx.enter_context(tc.tile_pool(name="ps", bufs=4, space="PSUM"))
        tpool = ctx.enter_context(tc.tile_pool(name="pt", bufs=2, space="PSUM"))
        qpool = ctx.enter_context(tc.tile_pool(name="pg", bufs=2, space="PSUM"))
        dram = ctx.enter_context(tc.tile_pool(name="dram", bufs=1, space="DRAM"))

        ident = const.tile([128, 128], F32)
        masks.make_identity(nc, ident[:])
        iota = const.tile([128, 128], vdt)
        nc.sync.dma_start(iota[:], d_iota[:])

        wt = {}
        for g in "ruc":
            W0 = const.tile([FW, H], F32, tag=f"W0{g}")
            W1 = const.tile([FW, H], F32, tag=f"W1{g}")
            W2 = const.tile([FW, H], F32, tag=f"W2{g}")
            nc.sync.dma_start(W0[:], d_W[g][0])
            nc.sync.dma_start(W1[:], d_W[g][1])
            nc.sync.dma_start(W2[:], d_W[g][2])
            WS = const.tile([FW + 2, H], F32, tag=f"WS{g}")
            nc.gpsimd.memset(WS[:], 0.0)
            nc.vector.tensor_tensor(WS[0:FW, :], W0[:], W2[:], OP.subtract)
            nc.sync.dma_start(WS[FW:FW + 1, :], d_b[g][:])
            WC = const.tile([FW, H], F32, tag=f"WC{g}")
            nc.vector.tensor_scalar(WC[:], W2[:], 2.0, None, OP.mult)
            wt[g] = (WS, W1, WC)

        # degree -> dinv, -dinv, -dinv^2 per step
        dinvs = []
        for t in range(n_steps):
            wel = sb.tile([128, NBLK, lell], F32, tag="wel")
            nc.sync.dma_start(wel[:], d_wel[t])
            deg = spool.tile([128, NBLK], F32, tag="deg")
            nc.vector.tensor_reduce(deg[:], wel[:], axis=mybir.AxisListType.X, op=OP.add)
            sq = spool.tile([128, NBLK], F32, tag="sq")
            nc.vector.tensor_scalar(sq[:], deg[:], 1e-30, None, OP.max)
            nc.scalar.activation(sq[:], sq[:], ACTF.Sqrt)
            rec = spool.tile([128, NBLK], F32, tag="rec")
            nc.vector.reciprocal(rec[:], sq[:])
            msk = spool.tile([128, NBLK], F32, tag="msk")
            nc.vector.tensor_scalar(msk[:], deg[:], 0.0, None, OP.is_gt)
            dv = const.tile([128, NBLK], F32, tag=f"dv{t}")
            nc.vector.tensor_tensor(dv[:], rec[:], msk[:], OP.mult)
            ndv = const.tile([128, NBLK], F32, tag=f"ndv{t}")
            nc.vector.tensor_scalar(ndv[:], dv[:], -1.0, None, OP.mult)
            nd2 = const.tile([128, NBLK], F32, tag=f"nd2{t}")
            nc.vector.tensor_tensor(nd2[:], dv[:], ndv[:], OP.mult)
            dinvs.append((dv, ndv, nd2))

        tabs = [
            dram.tile([CORES * NPAD, ROWE], vdt, tag=f"tab{i}", name=f"tab{i}")
            for i in range(4)
        ]
        bnc = [
            dram.tile([NPAD, ROWE], vdt, tag=f"bnc{i}", name=f"bnc{i}")
            for i in range(4)
        ]

        combA = const.tile([128, NBLK, FW + 2], F32, tag="combA")
        comb2A = const.tile([128, NBLK, FW + 2], F32, tag="comb2A")
        for cb in (combA, comb2A):
            nc.gpsimd.memset(cb[:], 0.0)
            nc.gpsimd.memset(cb[:, :, FW:FW + 1], 1.0)
        ustage = const.tile([128, NBLK, ROWE], vdt, tag="ustage")
        nc.gpsimd.memset(ustage[:], 0.0)
        Ubuf = const.tile([128, NBLK, H], F32, tag="Ubuf")
        hv = combA[:, :, DIN:FW]   # h lives in comb

        KDBG = os.environ.get("KDBG", "")

        def do_ag(i):
            nc.gpsimd.dma_start(
                bnc[i][:].rearrange("(b p) e -> p b e", p=128), ustage[:]
            )
            if KDBG == "nocc":
                nc.gpsimd.dma_start(tabs[i][0:NPAD, :], bnc[i][:])
                return
            nc.gpsimd.collective_compute(
                "AllGather", OP.bypass,
                replica_groups=[list(range(CORES))],
                ins=[bnc[i][:].opt()], outs=[tabs[i][:].opt()],
            )

        def scale_stage(srcA, scol, i):
            # ustage[:, b, 0:FW] = srcA[:, b, :] * scol[:, b] ; then DMA+AG
            for b in range(NBLK):
                nc.vector.tensor_scalar(
                    ustage[:, b, 0:FW], srcA[:, b, 0:FW], scol[:, b:b + 1], None, OP.mult
                )
            do_ag(i)

        SUB = 8  # chunks per dma_gather; 1024 descriptors fits the SWDGE ring

        def edge_pass(tab, Abuf, idx, dl, wv):
            for b in range(NBLK):
                g = gpool.tile([128, cblk, ROWE], vdt, tag="g")
                for s0 in range(0, cblk, SUB):
                    s1 = min(s0 + SUB, cblk)
                    nidx = (s1 - s0) * 128
                    nc.gpsimd.dma_gather(
                        g[:, s0:s1, :], tab[:],
                        idx[:, (b * cblk + s0) * 8:(b * cblk + s1) * 8],
                        num_idxs=nidx, num_idxs_reg=nidx, elem_size=ROWE,
                    )
                ps = ppool.tile([128, FW], F32, tag="acc")
                for cx in range(cblk):
                    col = b * cblk + cx
                    D = dpool.tile([128, 128], vdt, tag="D")
                    nc.vector.tensor_scalar(
                        D[:], iota[:], dl[:, col:col + 1], wv[:, col:col + 1],
                        OP.is_equal, OP.mult,
                    )
                    nc.tensor.matmul(
                        ps[:], D[:], g[:, cx, 0:FW],
                        start=(cx == 0), stop=(cx == cblk - 1),
                    )
                nc.scalar.copy(Abuf[:, b, :], ps[:])

        def tr(src_ap, fr):
            pt = tpool.tile([fr, 128], F32, tag="tp")
            nc.tensor.matmul(pt[:], src_ap, ident[:], is_transpose=True)
            s = sb.tile([fr, 128], F32, tag="tps")
            nc.scalar.copy(s[:], pt[:])
            return s

        def gate_mms(Tc, T1, T2, g, func, outt):
            WS, W1, WC = wt[g]
            psq = qpool.tile([128, H], F32, tag="gps")
            nc.tensor.matmul(psq[:], Tc[0:FW + 2, :], WS[:], start=True, stop=False)
            nc.tensor.matmul(psq[:], T1[0:FW, :], W1[:], start=False, stop=False)
            nc.tensor.matmul(psq[:], T2[0:FW, :], WC[:], start=False, stop=True)
            nc.scalar.activation(outt[:], psq[:], func)

        A1 = const.tile([128, NBLK, FW], F32, tag="A1")
        A2 = const.tile([128, NBLK, FW], F32, tag="A2")
        A1p = const.tile([128, NBLK, FW], F32, tag="A1p")
        A2p = const.tile([128, NBLK, FW], F32, tag="A2p")

        for t in range(n_steps):
            dv, ndv, nd2 = dinvs[t]
            nc.sync.dma_start(comb2A[:, :, 0:DIN], d_x[t])
            if t == 0:
                nc.sync.dma_start(combA[:, :, 0:DIN], d_x[t])
                scale_stage(combA, dv, 0)
            idx = sb.tile([128, nch * 8], I16, tag="idx")
            nc.sync.dma_start(idx[:], d_idx[t])
            dl = sb.tile([128, nch], F32, tag="dl")
            nc.sync.dma_start(dl[:], d_dl[t])
            wv = sb.tile([128, nch], F32, tag="wv")
            nc.sync.dma_start(wv[:], d_w[t])

            edge_pass(tabs[0], A1, idx, dl, wv)
            scale_stage(A1, nd2, 1)
            for b in range(NBLK):
                nc.vector.tensor_scalar(A1p[:, b, :], A1[:, b, :], ndv[:, b:b + 1], None, OP.mult)
            edge_pass(tabs[1], A2, idx, dl, wv)
            for b in range(NBLK):
                nc.vector.tensor_scalar(A2p[:, b, :], A2[:, b, :], ndv[:, b:b + 1], None, OP.mult)

            for b in range(NBLK):
                Tc = tr(combA[:, b, :], FW + 2)
                T1 = tr(A1p[:, b, :], FW)
                T2 = tr(A2p[:, b, :], FW)
                rb = spool.tile([128, H], F32, tag="rb")
                gate_mms(Tc, T1, T2, "r", ACTF.Sigmoid, rb)
                ub = Ubuf[:, b, :]
                psq = qpool.tile([128, H], F32, tag="gps")
                WS, W1, WC = wt["u"]
                nc.tensor.matmul(psq[:], Tc[0:FW + 2, :], WS[:], start=True, stop=False)
                nc.tensor.matmul(psq[:], T1[0:FW, :], W1[:], start=False, stop=False)
                nc.tensor.matmul(psq[:], T2[0:FW, :], WC[:], start=False, stop=True)
                nc.scalar.activation(ub, psq[:], ACTF.Sigmoid)
                nc.vector.tensor_tensor(comb2A[:, b, DIN:FW], rb[:], hv[:, b, :], OP.mult)

            scale_stage(comb2A, dv, 2)
            edge_pass(tabs[2], A1, idx, dl, wv)
            scale_stage(A1, nd2, 3)
            for b in range(NBLK):
                nc.vector.tensor_scalar(A1p[:, b, :], A1[:, b, :], ndv[:, b:b + 1], None, OP.mult)
            edge_pass(tabs[3], A2, idx, dl, wv)
            for b in range(NBLK):
                nc.vector.tensor_scalar(A2p[:, b, :], A2[:, b, :], ndv[:, b:b + 1], None, OP.mult)

            for b in range(NBLK):
                Tc2 = tr(comb2A[:, b, :], FW + 2)
                T1c = tr(A1p[:, b, :], FW)
                T2c = tr(A2p[:, b, :], FW)
                cb = spool.tile([128, H], F32, tag="cb")
                gate_mms(Tc2, T1c, T2c, "c", ACTF.Tanh, cb)
                tmp = spool.tile([128, H], F32, tag="tmp")
                nc.vector.tensor_tensor(tmp[:], hv[:, b, :], cb[:], OP.subtract)
                nc.vector.tensor_tensor(tmp[:], Ubuf[:, b, :], tmp[:], OP.mult)
                nc.vector.tensor_tensor(hv[:, b, :], cb[:], tmp[:], OP.add)

            if t < n_steps - 1:
                nc.sync.dma_start(combA[:, :, 0:DIN], d_x[t + 1])
                scale_stage(combA, dinvs[t + 1][0], 0)

        nc.sync.dma_start(d_out[:], combA[:, :, DIN:FW])
    nc.finalize()
    return nc


def kernel(x, edge_idx, edge_attr, Wr, br, Wu, bu, Wc, bc, n_steps=T_FULL, vdt=VDT, trace=False):
    maps, cblk, lell = preprocess(x, edge_idx, edge_attr, n_steps, vdt)
    iota = np.tile(np.arange(128, dtype=np.float32), (128, 1)).astype(_npdt(vdt))
    shared = dict(
        Wr=np.asarray(Wr, np.float32), Wu=np.asarray(Wu, np.float32),
        Wc=np.asarray(Wc, np.float32),
        br=np.asarray(br, np.float32).reshape(1, H),
        bu=np.asarray(bu, np.float32).reshape(1, H),
        bc=np.asarray(bc, np.float32).reshape(1, H),
        iota=iota,
    )
    in_maps = [{**m, **shared} for m in maps]
    nc = build(n_steps, cblk, lell, vdt)
    import time as _time
    res = run_bass_kernel_spmd(nc, in_maps, core_ids=list(range(CORES)), trace=trace)
    if os.environ.get("KREPEAT", "0") == "1":
        t0 = _time.perf_counter()
        res = run_bass_kernel_spmd(nc, in_maps, core_ids=list(range(CORES)), trace=trace)
        kernel.exec_wall_s = _time.perf_counter() - t0
    else:
        kernel.exec_wall_s = 0.0
    kernel.last_result = res
    outs = []
    for c in range(CORES):
        ho = res.results[c]["h_out"]            # [128, NBLK, H]
        outs.append(ho.transpose(1, 0, 2).reshape(NPAD, H)[:NLOC])
    return np.concatenate(outs, axis=0).astype(np.float32)


if __name__ == "__main__":
    pass

